# revision 1
# baseline (speedup 1.0000x reference)
"""Multi-head attention (B=4, S=2048, D=1024, H=16, Dh=64) on 8 TRN2 NeuronCores.

Sharding: core = (batch, head_group): 4 batches x 2 head-groups of 8 heads.
Fully data-parallel SPMD - no collectives.

v17: bf16 inputs from host (half DMA volume, no staging casts); one globally software-pipelined stream over all (q-block, quad, k-tile)
steps - scores for step i+1 are emitted before AV of step i, across quad
boundaries too, so the ScalarE exp stream never stalls on the PE FIFO.
Quad tails release their PSUM banks eagerly (DVE copies right after the last
AV) with the rest of the tail interleaved into the next quad. Phase 1 runs
the critical path (K0, Q0) on two parallel DMA queues (SP + Activation);
all other projections are filler chunks inside the attention stream through
a dedicated aux PSUM bank. All matmul operands bf16; masking via host-zeroed
V row + masked ones vector in the softmax-sum matmul.
"""

from contextlib import ExitStack

import ml_dtypes
import numpy as np

import concourse.bass as bass
import concourse.bacc as bacc
import concourse.mybir as mybir
import concourse.tile as tile
from concourse.bass_utils import run_bass_kernel_spmd
from concourse.masks import make_identity

B = 4
SEQ = 2048
DM = 1024
H = 16
DH = 64
NCORES = 8
CPC = 512          # output columns per core (8 heads x 64)
P = 128
NQB = SEQ // 512   # q blocks of 512
NKT = SEQ // P     # k tiles of 128
NDT = DM // P      # d_model tiles of 128

F32 = mybir.dt.float32
BF16 = mybir.dt.bfloat16
EXP = mybir.ActivationFunctionType.Exp

_compiled = None


def _emit(ctx: ExitStack, tc: tile.TileContext, qt, kt, vt, wq, wk, wv, bmask, out):
    nc = tc.nc

    small = ctx.enter_context(tc.tile_pool(name="small", bufs=1))
    wpool = ctx.enter_context(tc.tile_pool(name="wpool", bufs=1))
    stage4 = ctx.enter_context(tc.tile_pool(name="stage4", bufs=5))
    proj = ctx.enter_context(tc.tile_pool(name="proj", bufs=1))
    epool = ctx.enter_context(tc.tile_pool(name="epool", bufs=10))
    opool = ctx.enter_context(tc.tile_pool(name="opool", bufs=2))
    oparts = ctx.enter_context(tc.tile_pool(name="oparts", bufs=2))
    ps_sc = ctx.enter_context(tc.tile_pool(name="ps_sc", bufs=2, space="PSUM"))
    ps_ot = ctx.enter_context(tc.tile_pool(name="ps_ot", bufs=2, space="PSUM"))
    ps_sm = ctx.enter_context(tc.tile_pool(name="ps_sm", bufs=1, space="PSUM"))
    ps_aux = ctx.enter_context(tc.tile_pool(name="ps_aux", bufs=1, space="PSUM"))

    ident = small.tile([P, P], F32)
    make_identity(nc, ident[:])
    ident_bf = small.tile([P, P], BF16)
    nc.vector.tensor_copy(ident_bf[:], ident[:])
    mones_f = small.tile([P, NKT], F32)
    nc.sync.dma_start(mones_f[:], bmask.ap())
    mones = small.tile([P, NKT], BF16)
    nc.vector.tensor_copy(mones[:], mones_f[:])

    kt_r = kt.ap().rearrange("(dt p) q -> p dt q", p=P)
    vt_r = vt.ap().rearrange("(dt p) q -> p dt q", p=P)
    qt_r = qt.ap().rearrange("(dt p) q -> p dt q", p=P)

    w_sb = {}

    def load_w(name, w, eng):
        t = wpool.tile([P, NDT, CPC], BF16, tag=name, name=name)
        eng.dma_start(t[:], w.ap().rearrange("(dt p) c -> p dt c", p=P))
        w_sb[name] = t

    kproj = [proj.tile([P, SEQ], BF16, tag=f"kproj{p}", name=f"kproj{p}")
             for p in range(4)]
    qproj = [[proj.tile([P, 512], BF16, tag=f"qproj{p}_{qb}", name=f"qproj{p}_{qb}")
              for qb in range(NQB)] for p in range(4)]
    v_t = [proj.tile([P, 512], BF16, tag=f"v{k}", name=f"v{k}") for k in range(NKT)]

    def stage_block(src_r, blk, nm, eng):
        st = stage4.tile([P, NDT, 512], BF16, tag="st", name=f"st_{nm}")
        eng.dma_start(st[:], src_r[:, :, blk * 512:(blk + 1) * 512])
        return st

    def kq_chunk(wname, st, dst, p, pool):
        ps = pool.tile([P, 512], F32, tag="scores" if pool is ps_sc else "aux")
        for dt in range(NDT):
            nc.tensor.matmul(
                ps[:],
                w_sb[wname][:, dt, 128 * p:128 * (p + 1)],
                st[:, dt, :],
                start=(dt == 0),
                stop=(dt == NDT - 1),
            )
        nc.vector.tensor_copy(dst[:], ps[:])

    def v_chunk(st, kt_i, pool):
        sub = kt_i % 4
        ps = pool.tile([P, 512], F32, tag="scores" if pool is ps_sc else "aux")
        for dt in range(NDT):
            nc.tensor.matmul(
                ps[:],
                st[:, dt, 128 * sub:128 * (sub + 1)],
                w_sb["wv"][:, dt, :],
                start=(dt == 0),
                stop=(dt == NDT - 1),
            )
        nc.vector.tensor_copy(v_t[kt_i][:], ps[:])

    # ---- attention stream helpers ---------------------------------------
    quad_state = {}
    pend = {}

    def emit_scores(step):
        qb, quad, kt_i = step
        pairs = (2 * quad, 2 * quad + 1)
        e_tiles = []
        for pi, pr in enumerate(pairs):
            st_ps = ps_sc.tile([P, 1024], F32, tag="scores")
            for hh in range(2):
                rows = slice(64 * hh, 64 * (hh + 1))
                nc.tensor.matmul(
                    st_ps[:, 512 * hh:512 * (hh + 1)],
                    kproj[pr][rows, kt_i * P:(kt_i + 1) * P],
                    qproj[pr][qb][rows, :],
                    start=True,
                    stop=True,
                    tile_position=(64 * hh, 0),
                )
            e = epool.tile([P, 1024], BF16, tag="e")
            nc.scalar.activation(e[:], st_ps[:], EXP, scale=0.125)
            e_tiles.append(e)
        pend[step] = e_tiles

    def emit_av(step):
        qb, quad, kt_i = step
        pairs = (2 * quad, 2 * quad + 1)
        if kt_i == 0:
            quad_state[(qb, quad)] = (
                [ps_ot.tile([P, 512], F32, tag="ot", name=f"ot{qb}_{quad}_{i}")
                 for i in range(2)],
                ps_sm.tile([P, 512], F32, tag="sums", name=f"sm{qb}_{quad}"),
            )
        ot_ps, sm_ps = quad_state[(qb, quad)]
        e_tiles = pend.pop(step)
        for pi, pr in enumerate(pairs):
            e = e_tiles[pi]
            for hh in range(2):
                cols = slice(128 * pr + 64 * hh, 128 * pr + 64 * (hh + 1))
                nc.tensor.matmul(
                    ot_ps[pi][64 * hh:64 * (hh + 1), :],
                    v_t[kt_i][:, cols],
                    e[:, 512 * hh:512 * (hh + 1)],
                    start=(kt_i == 0),
                    stop=(kt_i == NKT - 1),
                    tile_position=(0, 64 * hh),
                    skip_group_check=(hh == 1),
                )
        for j in range(4):
            nc.tensor.matmul(
                sm_ps[32 * j:32 * j + 1, :],
                mones[:, kt_i:kt_i + 1],
                e_tiles[j // 2][:, 512 * (j % 2):512 * (j % 2 + 1)],
                start=(kt_i == 0),
                stop=(kt_i == NKT - 1),
                tile_position=(0, 32 * j),
                skip_group_check=(j > 0),
            )

    def make_tail(qb, quad):
        ot_ps, sm_ps = quad_state.pop((qb, quad))
        st = {}

        def t0():
            # free sm + ot banks ASAP (DVE copies only)
            sums_sb = opool.tile([P, 512], F32, tag="sums_sb",
                                 name=f"ssb{qb}_{quad}")
            nc.vector.memset(sums_sb[:], 1.0)
            for j in range(4):
                nc.vector.tensor_copy(
                    sums_sb[32 * j:32 * j + 1, :], sm_ps[32 * j:32 * j + 1, :]
                )
            ot_sb = [opool.tile([P, 512], BF16, tag="ot_sb",
                                name=f"otsb{qb}_{quad}_{i}") for i in range(2)]
            for pi in range(2):
                nc.vector.tensor_copy(ot_sb[pi][:], ot_ps[pi][:])
            st["sums_sb"] = sums_sb
            st["ot_sb"] = ot_sb

        def t1():
            rcp = opool.tile([P, 16], F32, tag="rcp", name=f"rcp{qb}_{quad}")
            for c in range(4):
                tr_s = ps_aux.tile([P, P], F32, tag="aux", name=f"trs{qb}_{quad}_{c}")
                nc.tensor.transpose(tr_s[:], st["sums_sb"][:, c * P:(c + 1) * P],
                                    ident[:])
                for j in range(4):
                    nc.vector.reciprocal(
                        rcp[:, 4 * c + j:4 * c + j + 1], tr_s[:, 32 * j:32 * j + 1]
                    )
            st["rcp"] = rcp
            st["o_part"] = oparts.tile(
                [P, 4, 256], F32, tag="opart", name=f"opart{qb}_{quad}"
            )

        def t_pi(pi):
            o_part, rcp = st["o_part"], st["rcp"]
            for c in range(4):
                tr_o = ps_aux.tile([P, P], BF16, tag="aux",
                                   name=f"tro{qb}_{quad}_{pi}_{c}")
                nc.tensor.transpose(tr_o[:], st["ot_sb"][pi][:, c * P:(c + 1) * P],
                                    ident_bf[:])
                for hh in range(2):
                    lh = 2 * pi + hh
                    nc.vector.tensor_scalar(
                        o_part[:, c, 64 * lh:64 * (lh + 1)],
                        tr_o[:, 64 * hh:64 * (hh + 1)],
                        rcp[:, 4 * c + lh:4 * c + lh + 1],
                        None,
                        mybir.AluOpType.mult,
                    )

        def t_out():
            for c in range(4):
                nc.sync.dma_start(
                    out.ap()[
                        qb * 512 + c * P:qb * 512 + (c + 1) * P,
                        quad * 256:(quad + 1) * 256,
                    ],
                    st["o_part"][:, c, :],
                )

        return t0, [t1, lambda: t_pi(0), lambda: t_pi(1), t_out]

    # ---- phase 1: K/V/Q block 0 (SP queue), then start the stream ------
    load_w("wk", wk, nc.sync)
    st_k0 = stage_block(kt_r, 0, "k0", nc.sync)
    load_w("wq", wq, nc.sync)
    st_q0 = stage_block(qt_r, 0, "q0", nc.sync)
    for p in range(4):
        kq_chunk("wk", st_k0, kproj[p][:, 0:512], p, ps_sc)
    for p in range(4):
        kq_chunk("wq", st_q0, qproj[p][0][:], p, ps_sc)
    load_w("wv", wv, nc.sync)
    st_v0 = stage_block(vt_r, 0, "v0", nc.sync)
    vstate = {0: st_v0}
    kstate = {1: stage_block(kt_r, 1, "k1", nc.sync)}
    qstate = {}

    steps = [(qb, quad, k) for qb in range(NQB) for quad in (0, 1)
             for k in range(NKT)]

    # ---- filler schedule (keyed by global step index), v4 layout --------
    fillers = {}

    def add(i, fn):
        fillers.setdefault(i, []).append(fn)

    def v_stage(kb, eng):
        def f():
            vstate[kb] = stage_block(vt_r, kb, f"v{kb}", eng)
        return f

    def k_stage(kb, eng):
        def f():
            kstate[kb] = stage_block(kt_r, kb, f"k{kb}", eng)
        return f

    def mkv(kt_i):
        def f():
            v_chunk(vstate[kt_i // 4], kt_i, ps_aux)
        return f

    def mkk(kb, p):
        def f():
            kq_chunk("wk", kstate[kb], kproj[p][:, kb * 512:(kb + 1) * 512],
                     p, ps_aux)
        return f

    add(0, v_stage(1, nc.sync))
    add(4, v_stage(2, nc.sync))
    add(8, v_stage(3, nc.sync))
    for kt_i in range(4, 16):
        add(kt_i - 1, mkv(kt_i))
    add(1, k_stage(2, nc.sync))
    add(5, k_stage(3, nc.sync))
    for kb in (1, 2, 3):
        for p in range(4):
            add(4 * (kb - 1) + p, mkk(kb, p))

    def q_stage(qb):
        def f():
            qstate[qb] = stage_block(qt_r, qb, f"q{qb}", nc.sync)
        return f

    def mkq(qb, p):
        def f():
            kq_chunk("wq", qstate[qb], qproj[p][qb][:], p, ps_aux)
        return f

    for qb in range(1, NQB):
        base = (2 * qb - 1) * 16
        add(base + 0, q_stage(qb))
        for p in range(4):
            add(base + 2 + 4 * p, mkq(qb, p))

    # ---- the pipelined stream -------------------------------------------
    emit_scores(steps[0])
    for kt_i in range(4):
        v_chunk(st_v0, kt_i, ps_aux)
    for i, step in enumerate(steps):
        if i + 1 < len(steps):
            emit_scores(steps[i + 1])
        emit_av(step)
        qb, quad, kt_i = step
        if kt_i == NKT - 1:
            t0, rest = make_tail(qb, quad)
            t0()
            if i + 1 < len(steps):
                for j, piece in enumerate(rest):
                    add(i + 1 + j, piece)
            else:
                for piece in rest:
                    piece()
        for fn in fillers.get(i, ()):
            fn()


def build():
    global _compiled
    if _compiled is not None:
        return _compiled
    nc = bacc.Bacc("TRN2", target_bir_lowering=False, debug=False)
    qt = nc.dram_tensor("qt", [DM, SEQ], BF16, kind="ExternalInput")
    kt = nc.dram_tensor("kt", [DM, SEQ], BF16, kind="ExternalInput")
    vt = nc.dram_tensor("vt", [DM, SEQ], BF16, kind="ExternalInput")
    wq = nc.dram_tensor("wq", [DM, CPC], BF16, kind="ExternalInput")
    wk = nc.dram_tensor("wk", [DM, CPC], BF16, kind="ExternalInput")
    wv = nc.dram_tensor("wv", [DM, CPC], BF16, kind="ExternalInput")
    bmask = nc.dram_tensor("bmask", [P, NKT], F32, kind="ExternalInput")
    out = nc.dram_tensor("out", [SEQ, CPC], F32, kind="ExternalOutput")
    with tile.TileContext(nc) as tc:
        with ExitStack() as ctx:
            _emit(ctx, tc, qt, kt, vt, wq, wk, wv, bmask, out)
    nc.compile()
    _compiled = nc
    return nc


def make_in_maps(Q_seq, K_seq, V_seq, V_len, WQ, WK, WV):
    in_maps = []
    for core in range(NCORES):
        b, hg = divmod(core, 2)
        cols = slice(hg * CPC, (hg + 1) * CPC)
        vl = int(V_len[b, 0])
        bm = np.ones((P, NKT), np.float32)
        bm[vl % P, vl // P] = 0.0
        vt_m = np.ascontiguousarray(V_seq[b].T)
        vt_m[:, vl] = 0.0
        bf = ml_dtypes.bfloat16
        in_maps.append(
            {
                "qt": np.ascontiguousarray(Q_seq[b].T).astype(bf),
                "kt": np.ascontiguousarray(K_seq[b].T).astype(bf),
                "vt": vt_m.astype(bf),
                "wq": np.ascontiguousarray(WQ[:, cols]).astype(bf),
                "wk": np.ascontiguousarray(WK[:, cols]).astype(bf),
                "wv": np.ascontiguousarray(WV[:, cols]).astype(bf),
                "bmask": bm,
            }
        )
    return in_maps


def kernel(Q_seq, K_seq, V_seq, Q_len, V_len, WQ, WK, WV, _trace=False):
    nc = build()
    in_maps = make_in_maps(Q_seq, K_seq, V_seq, V_len, WQ, WK, WV)
    res = run_bass_kernel_spmd(
        nc, in_maps, core_ids=list(range(NCORES)), trace=_trace
    )
    out = np.empty((B, SEQ, H * DH), np.float32)
    for core in range(NCORES):
        b, hg = divmod(core, 2)
        out[b, :, hg * CPC:(hg + 1) * CPC] = res.results[core]["out"]
    for b in range(B):
        out[b, int(Q_len[b, 0]), :] = 0.0
    if _trace:
        kernel._last_results = res
    return out

